# revision 15
# baseline (speedup 1.0000x reference)
"""Trainium2 Bass kernel for CapsNet dynamic routing (nn_Capsule_34342558498916).

Full inputs:  u_vecs (64, 64, 1024) f32, W (1024, 32, 64, 16) f32
Full output:  (64, 16, 32) f32  == transpose(v, (0, 2, 1)) of v (B, N, D)

Sharding: capsule dim N=32 split across 8 cores (4 capsules each); routing
(softmax over u) is fully core-local, so no collectives.

Per-core design (v2):
  Phase 1 einsum as K=128 matmuls: stationary = block-diagonal u-vectors
  [(par,c), (par,b)] (par = u-parity), moving = W rows [(par,c), (d,n)].
  One matmul per u-pair instead of two K=64 quadrant matmuls: halves PE
  streaming cost.  The block-diag stationary is assembled by strided DMA
  from a host layout chosen so both src and dst runs are >= 512B (avoids
  the DMA read-modify-write penalty); off-diagonal zeros are memset once
  and never rewritten (buffer ring reuses the same diag windows).
  Optionally (PH1_W16) W is shipped fp16 and u as an exact fp16 hi+lo
  split: two fp16 matmuls per u-pair accumulate in PSUM (same PE cost as
  one fp32r pass, W-rounding error ~9e-3, DMA bytes halved).

  Routing contractions run on the PE at 1 cycle/row via float32r moving
  operands with out-free >= 256 (4x the fp32 rate):
    s1 = sum_u u_hat        : fold-stationary slot-accumulate, free=512
    b += sum_d u_hat * v    : identity-stationary accumulate over d,
                              free = (64 t-pairs, 4 n) = 256
    s~ = sum_u e * u_hat    : fp16 product fold (as baseline), free=512
  Elementwise products split DVE/Pool; exp on ACT with per-partition
  -rowmax bias; logits b kept f32 in SBUF.
"""

import sys

import numpy as np

for _p in ("/opt/trn_rl_repo", "/opt/pypackages"):
    if _p not in sys.path:
        sys.path.append(_p)

import concourse.bass as bass
from concourse import bacc
import concourse.mybir as mybir
from concourse import tile
from concourse.bass_utils import run_bass_kernel_spmd

# Problem dims (hardcoded per harness contract)
B, C, U, N, D = 64, 64, 1024, 32, 16
NCORES = 8
NL = N // NCORES          # 4 capsules per core
T = U // 2                # 512 u-pairs
DN = D * NL               # 64 = phase-1 moving free dim (d, n4)
P = 128
EPS = 1e-8
ROUTINGS = 3

TCH = 16                  # u-pairs per phase-1 chunk
NCH = T // TCH            # 32 chunks
UBUF = 3                  # utbd stationary ring slots
TC = 64                   # u-pairs per routing chunk
NRC = T // TC             # 8 routing chunks
DH = D // 2               # d-half for pr staging

dt = mybir.dt
AF = mybir.ActivationFunctionType
ALU = mybir.AluOpType

_COMPILED = {}

# Variant flags
PH1_W16 = False   # fp16 W + fp16 hi/lo u stationaries (halves DMA, ~9e-3 err)
S1_PE = False      # s1 via PE f32r fold (else DVE/Pool reduces)
BUPD_PE = True    # b-update d-sum via PE f32r matmuls (else DVE/Pool reduce)


def _squash_core(nc, sm, s_red_ap, zi_or_scale, s_sb, v16_dst):
    """s_red_ap: un-normalized s~ [B, D, NL]; s = s~ * zi or * scalar;
    v = squash(s)."""
    if isinstance(zi_or_scale, float):
        nc.vector.tensor_scalar_mul(s_sb[:], s_red_ap, zi_or_scale)
    else:
        zi_bc = zi_or_scale[:].unsqueeze(1).broadcast_to([B, D, NL])
        nc.vector.tensor_mul(s_sb[:], s_red_ap, zi_bc)
    ssq = sm.tile([B, D, NL], dt.float32, tag="ssq")
    nc.vector.tensor_mul(ssq[:], s_sb[:], s_sb[:])
    s2 = sm.tile([B, NL], dt.float32, tag="s2")
    nc.vector.tensor_reduce(
        s2[:], ssq[:].transpose([0, 2, 1]), axis=mybir.AxisListType.X, op=ALU.add
    )
    s2e = sm.tile([B, NL], dt.float32, tag="s2e")
    nc.vector.tensor_scalar_add(s2e[:], s2[:], EPS)
    rt = sm.tile([B, NL], dt.float32, tag="rt")
    nc.scalar.activation(rt[:], s2e[:], AF.Sqrt)
    den = sm.tile([B, NL], dt.float32, tag="den")
    nc.vector.tensor_scalar_add(den[:], s2e[:], 1.0)
    deni = sm.tile([B, NL], dt.float32, tag="deni")
    nc.vector.reciprocal(deni[:], den[:])
    f = sm.tile([B, NL], dt.float32, tag="f")
    nc.vector.tensor_mul(f[:], rt[:], deni[:])
    v_sb = sm.tile([B, D, NL], dt.float32, tag="v_sb")
    nc.vector.tensor_mul(v_sb[:], s_sb[:], f[:].unsqueeze(1).broadcast_to([B, D, NL]))
    nc.vector.tensor_copy(v16_dst, v_sb[:])
    return v_sb


def _squash_and_v(nc, sm, s_ps8, zi_or_scale, s_sb, v16_dst):
    """Slotted PSUM accumulator [B, 8, D, NL] -> fold slots -> squash."""
    s_red = sm.tile([B, D, NL], dt.float32, tag="s_red")
    nc.vector.tensor_reduce(
        s_red[:], s_ps8[:].transpose([0, 2, 3, 1]),
        axis=mybir.AxisListType.X, op=ALU.add,
    )
    return _squash_core(nc, sm, s_red[:], zi_or_scale, s_sb, v16_dst)


def _build_program():
    nc = bacc.Bacc()

    st_dt = dt.float16 if PH1_W16 else dt.float32
    mv_dt = dt.float16 if PH1_W16 else dt.float32

    # DRAM inputs.  ut_*_r layout: [c(64), chunk, b(64), tin(TCH)] so that both
    # the DRAM source run (b*tin = 4KB) and the SBUF dest run (64 M-rows * TCH)
    # are contiguous.
    if PH1_W16:
        ut_hi0 = nc.dram_tensor("ut_hi0", [64, NCH, B * TCH], dt.float16, kind="ExternalInput")
        ut_hi1 = nc.dram_tensor("ut_hi1", [64, NCH, B * TCH], dt.float16, kind="ExternalInput")
        ut_lo0 = nc.dram_tensor("ut_lo0", [64, NCH, B * TCH], dt.float16, kind="ExternalInput")
        ut_lo1 = nc.dram_tensor("ut_lo1", [64, NCH, B * TCH], dt.float16, kind="ExternalInput")
        wt_d = nc.dram_tensor("wt16", [P, T, DN], dt.float16, kind="ExternalInput")
    else:
        ut_r0 = nc.dram_tensor("ut_r0", [64, NCH, B * TCH], dt.float32, kind="ExternalInput")
        ut_r1 = nc.dram_tensor("ut_r1", [64, NCH, B * TCH], dt.float32, kind="ExternalInput")
        wt_d = nc.dram_tensor("wt", [P, T, DN], dt.float32, kind="ExternalInput")
    ident_d = nc.dram_tensor("identr", [P, P], dt.float32r, kind="ExternalInput")
    fold16_d = nc.dram_tensor("fold16", [P, B], dt.float16, kind="ExternalInput")
    foldr_d = nc.dram_tensor("foldr", [P, B], dt.float32r, kind="ExternalInput")
    out_d = nc.dram_tensor("out", [B, D, NL], dt.float32, kind="ExternalOutput")

    with tile.TileContext(nc) as tc:
        with (
            tc.tile_pool(name="big", bufs=1) as big,
            tc.tile_pool(name="wts", bufs=3) as wts,
            tc.tile_pool(name="prp", bufs=2) as prp,
            tc.tile_pool(name="pr2p", bufs=2) as pr2p,
            tc.tile_pool(name="sm", bufs=1) as sm,
            tc.tile_pool(name="psB", bufs=2, space="PSUM") as psB,
            tc.tile_pool(name="psS", bufs=1, space="PSUM") as psS,
        ):
            # u_hat stored float32r so PE contractions stream it at 1 cyc/row
            u_hat = big.tile([P, T, D, NL], dt.float32r, tag="u_hat")
            e_sb = big.tile([P, T, NL], dt.float16, tag="e_sb")
            b_sb = big.tile([P, T, NL], dt.float32, tag="b_sb")
            # block-diag stationary ring: [P, slot, parity-block, b*t(flat)]
            # The matmul M-dim AP (stride TCH, count 128, offset tl) walks
            # both flat blocks contiguously; off-parity blocks stay zero.
            nst = 2 if PH1_W16 else 1
            utbd = big.tile([P, UBUF * nst, 2, B * TCH], st_dt, tag="utbd")
            ident = sm.tile([P, P], dt.float32r, tag="ident")
            fold16 = sm.tile([P, B], dt.float16, tag="fold16")
            foldr = sm.tile([P, B], dt.float32r, tag="foldr")
            vbc = sm.tile([P, D, NL], dt.float32, tag="vbc")
            mneg = sm.tile([P, NL], dt.float32, tag="mneg")
            bmax = sm.tile([P, NL], dt.float32, tag="bmax")
            bmax_p = sm.tile([P, NRC, NL], dt.float32, tag="bmax_p")
            tmp64 = sm.tile([B, NL], dt.float32, tag="tmp64")
            z_p = sm.tile([P, NL], dt.float32, tag="z_p")
            z_f = sm.tile([B, NL], dt.float32, tag="z_f")
            zi = sm.tile([B, NL], dt.float32, tag="zi")
            s_sb = sm.tile([B, D, NL], dt.float32, tag="s_sb")

            nc.sync.dma_start(ident[:], ident_d[:])
            nc.sync.dma_start(fold16[:], fold16_d[:])
            nc.sync.dma_start(foldr[:], foldr_d[:])

            # zero the off-parity windows of every utbd slot once
            for sl in range(UBUF * nst):
                nc.vector.memset(utbd[0:64, sl, 1, :], 0.0)
                nc.gpsimd.memset(utbd[64:128, sl, 0, :], 0.0)

            # ---------------- Phase 1: u_hat = einsum over c ----------------
            with tc.tile_pool(name="pp", bufs=3, space="PSUM") as pp:
                for ch in range(NCH):
                    sl = ch % UBUF
                    wt_ch = wts.tile([P, TCH, DN], mv_dt, tag="wt_ch")
                    nc.sync.dma_start(wt_ch[:], wt_d[:, ch * TCH:(ch + 1) * TCH, :])
                    if PH1_W16:
                        nc.scalar.dma_start(utbd[0:64, 2 * sl, 0, :], ut_hi0[:, ch, :])
                        nc.gpsimd.dma_start(utbd[64:128, 2 * sl, 1, :], ut_hi1[:, ch, :])
                        nc.scalar.dma_start(utbd[0:64, 2 * sl + 1, 0, :], ut_lo0[:, ch, :])
                        nc.gpsimd.dma_start(utbd[64:128, 2 * sl + 1, 1, :], ut_lo1[:, ch, :])
                    else:
                        nc.scalar.dma_start(utbd[0:64, sl, 0, :], ut_r0[:, ch, :])
                        nc.gpsimd.dma_start(utbd[64:128, sl, 1, :], ut_r1[:, ch, :])
                    for g in range(TCH // 8):
                        ps = pp.tile([P, 8, DN], dt.float32, tag="pp")
                        for j in range(8):
                            tl = g * 8 + j
                            if PH1_W16:
                                st0 = utbd[:, 2 * sl, :, :].rearrange(
                                    "p q (m t) -> p (q m) t", t=TCH)
                                st1 = utbd[:, 2 * sl + 1, :, :].rearrange(
                                    "p q (m t) -> p (q m) t", t=TCH)
                                nc.tensor.matmul(
                                    ps[:, j, :], st0[:, :, tl],
                                    wt_ch[:, tl, :], start=True, stop=False,
                                )
                                nc.tensor.matmul(
                                    ps[:, j, :], st1[:, :, tl],
                                    wt_ch[:, tl, :], start=False, stop=True,
                                )
                            else:
                                st = utbd[:, sl, :, :].rearrange(
                                    "p q (m t) -> p (q m) t", t=TCH)
                                nc.tensor.matmul(
                                    ps[:, j, :], st[:, :, tl],
                                    wt_ch[:, tl, :], start=True, stop=True,
                                )
                        t0 = ch * TCH + g * 8
                        dst = u_hat[:, t0:t0 + 8, :, :]
                        src = ps[:].rearrange("p e (d n) -> p e d n", d=D)
                        eng = (2 * ch + g) % 3
                        if eng == 2:
                            nc.scalar.copy(dst, src)
                        else:
                            nc.vector.tensor_copy(dst, src)

            # ---------------- Iteration 1: uniform c -> v1 ----------------
            if S1_PE:
                s_ps1 = psS.tile([B, 8, D, NL], dt.float32, tag="s_ps1")
                for q in range(T // 8):
                    nc.tensor.matmul(
                        s_ps1[:], foldr[:], u_hat[:, q * 8:(q + 1) * 8, :, :],
                        start=(q == 0), stop=(q == T // 8 - 1),
                    )
                v1_sb = _squash_and_v(nc, sm, s_ps1, 1.0 / U, s_sb,
                                      vbc[0:64, :, :])
            else:
                s1acc = sm.tile([P, 4, DN], dt.float32, tag="s1acc")
                for q4 in range(4):
                    eng = nc.vector
                    eng.tensor_reduce(
                        s1acc[:, q4, :],
                        u_hat[:, q4 * 128:(q4 + 1) * 128, :, :]
                            .transpose([0, 2, 3, 1]),
                        axis=mybir.AxisListType.X, op=ALU.add,
                    )
                s1red = sm.tile([P, DN], dt.float32, tag="s1red")
                nc.vector.tensor_reduce(
                    s1red[:], s1acc[:].transpose([0, 2, 1]),
                    axis=mybir.AxisListType.X, op=ALU.add,
                )
                s1tmp = sm.tile([B, DN], dt.float32, tag="s1tmp")
                nc.sync.dma_start(s1tmp[:], s1red[64:128, :])
                s1f = sm.tile([B, DN], dt.float32, tag="s1f")
                nc.vector.tensor_add(s1f[:], s1red[0:64, :], s1tmp[:])
                v1_sb = _squash_core(
                    nc, sm, s1f[:].rearrange("b (d n) -> b d n", d=D),
                    1.0 / U, s_sb, vbc[0:64, :, :],
                )
            nc.sync.dma_start(vbc[64:128, :, :], vbc[0:64, :, :])

            # ---------------- Iterations 2..3 ----------------
            for it in range(1, ROUTINGS):
                # b += sum_d u_hat * v
                for rc in range(NRC):
                    tb = rc * TC
                    if BUPD_PE:
                        bu = psB.tile([P, TC, NL], dt.float32, tag="bu")
                        for dh in range(2):
                            pr = prp.tile([P, TC, DH, NL], dt.float32r, tag="pr")
                            mul_eng = nc.vector if (2 * rc + dh) % 2 == 0 else nc.gpsimd
                            mul_eng.tensor_mul(
                                pr[:], u_hat[:, tb:tb + TC, dh * DH:(dh + 1) * DH, :],
                                vbc[:, dh * DH:(dh + 1) * DH, :].unsqueeze(1)
                                    .broadcast_to([P, TC, DH, NL]),
                            )
                            for d in range(DH):
                                nc.tensor.matmul(
                                    bu[:], ident[:], pr[:, :, d, :],
                                    start=(dh == 0 and d == 0),
                                    stop=(dh == 1 and d == DH - 1),
                                )
                        dst = b_sb[:, tb:tb + TC, :]
                        if it == 1:
                            nc.vector.tensor_copy(dst, bu[:])
                        else:
                            nc.vector.tensor_add(dst, dst, bu[:])
                    else:
                        pr = prp.tile([P, TC, D, NL], dt.float32, tag="prf")
                        mul_eng = nc.vector if rc % 2 == 0 else nc.gpsimd
                        mul_eng.tensor_mul(
                            pr[:], u_hat[:, tb:tb + TC, :, :],
                            vbc[:].unsqueeze(1).broadcast_to([P, TC, D, NL]),
                        )
                        red = sm.tile([P, TC, NL], dt.float32, tag="bu_dve")
                        red_eng = nc.vector
                        red_eng.tensor_reduce(
                            red[:], pr[:].transpose([0, 1, 3, 2]),
                            axis=mybir.AxisListType.X, op=ALU.add,
                        )
                        dst = b_sb[:, tb:tb + TC, :]
                        if it == 1:
                            nc.vector.tensor_copy(dst, red[:])
                        else:
                            nc.vector.tensor_add(dst, dst, red[:])
                    nc.vector.tensor_reduce(
                        bmax_p[:, rc, :],
                        b_sb[:, tb:tb + TC, :].transpose([0, 2, 1]),
                        axis=mybir.AxisListType.X, op=ALU.max,
                    )
                # row max over u (exp stability)
                nc.vector.tensor_reduce(
                    bmax[:], bmax_p[:].transpose([0, 2, 1]),
                    axis=mybir.AxisListType.X, op=ALU.max,
                )
                nc.sync.dma_start(tmp64[:], bmax[64:128, :])
                nc.vector.tensor_tensor(bmax[0:64, :], bmax[0:64, :], tmp64[:], op=ALU.max)
                nc.vector.tensor_scalar_mul(mneg[0:64, :], bmax[0:64, :], -1.0)
                nc.sync.dma_start(mneg[64:128, :], mneg[0:64, :])

                # e = exp(b - rowmax) on ACT
                for j in range(NL):
                    nc.scalar.activation(
                        e_sb[:, :, j], b_sb[:, :, j], AF.Exp,
                        bias=mneg[:, j:j + 1], scale=1.0,
                    )
                # Z = sum_u e
                nc.vector.tensor_reduce(
                    z_p[:], e_sb[:].transpose([0, 2, 1]),
                    axis=mybir.AxisListType.X, op=ALU.add,
                )
                nc.sync.dma_start(tmp64[:], z_p[64:128, :])
                nc.vector.tensor_tensor(z_f[:], z_p[0:64, :], tmp64[:], op=ALU.add)
                nc.vector.reciprocal(zi[:], z_f[:])

                # s~ = sum_u e * u_hat (fp16 products, PE fold-accumulate)
                s_ps = psS.tile([B, 8, D, NL], dt.float32, tag="s_ps")
                TC2 = 32
                for rc in range(T // TC2):
                    tb = rc * TC2
                    pr2 = pr2p.tile([P, TC2, D, NL], dt.float16, tag="pr2")
                    mul_eng = nc.vector if rc % 2 == 0 else nc.gpsimd
                    mul_eng.tensor_mul(
                        pr2[:], u_hat[:, tb:tb + TC2, :, :],
                        e_sb[:, tb:tb + TC2, :].unsqueeze(2)
                            .broadcast_to([P, TC2, D, NL]),
                    )
                    for g in range(TC2 // 8):
                        nc.tensor.matmul(
                            s_ps[:], fold16[:], pr2[:, g * 8:(g + 1) * 8, :, :],
                            start=(rc == 0 and g == 0),
                            stop=(rc == T // TC2 - 1 and g == TC2 // 8 - 1),
                        )
                v_sb = _squash_and_v(nc, sm, s_ps, zi, s_sb, vbc[0:64, :, :])
                if it < ROUTINGS - 1:
                    nc.sync.dma_start(vbc[64:128, :, :], vbc[0:64, :, :])

            nc.sync.dma_start(out_d[:], v_sb[:])

    nc.finalize()
    return nc


def _prep_inputs(u_vecs, W):
    """Host-side shard + relayout.  Returns per-core input maps."""
    u32 = np.ascontiguousarray(u_vecs, dtype=np.float32)
    # per u-parity p: [c, t, b] -> [c, chunk, b, tin]
    utc = u32.transpose(1, 2, 0).reshape(C, T, 2, B)           # c, t, par, b
    def chunked(par_arr):  # [c, t, b] -> [c, NCH, b*TCH]
        a = par_arr.reshape(C, NCH, TCH, B).transpose(0, 1, 3, 2)
        return np.ascontiguousarray(a).reshape(C, NCH, B * TCH)
    ut_par = [chunked(utc[:, :, p, :]) for p in range(2)]      # f32
    ident = np.eye(P, dtype=np.float32)
    fold = np.tile(np.eye(B, dtype=np.float32), (2, 1))        # [128, 64]
    common = {"identr": ident, "fold16": fold.astype(np.float16), "foldr": fold}
    if PH1_W16:
        ut_hi = [a.astype(np.float16) for a in ut_par]
        ut_lo = [(a - h.astype(np.float32)).astype(np.float16)
                 for a, h in zip(ut_par, ut_hi)]
        common.update({"ut_hi0": ut_hi[0], "ut_hi1": ut_hi[1],
                       "ut_lo0": ut_lo[0], "ut_lo1": ut_lo[1]})
    else:
        common.update({"ut_r0": ut_par[0], "ut_r1": ut_par[1]})
    in_maps = []
    Wf = np.ascontiguousarray(W, dtype=np.float32)
    for k in range(NCORES):
        wk = Wf[:, k * NL:(k + 1) * NL]                        # [U, NL, C, D]
        # [(par,c), t, (d, n4)]
        wkt = wk.transpose(0, 2, 3, 1).reshape(T, 2, C, D * NL)  # t, par, c, dn
        wt2 = np.ascontiguousarray(wkt.transpose(1, 2, 0, 3)).reshape(P, T, DN)
        m = dict(common)
        if PH1_W16:
            m["wt16"] = wt2.astype(np.float16)
        else:
            m["wt"] = wt2
        in_maps.append(m)
    return in_maps


def kernel(u_vecs: np.ndarray, W: np.ndarray) -> np.ndarray:
    if "nc" not in _COMPILED:
        _COMPILED["nc"] = _build_program()
    nc = _COMPILED["nc"]
    in_maps = _prep_inputs(np.asarray(u_vecs), np.asarray(W))
    res = run_bass_kernel_spmd(nc, in_maps, list(range(NCORES)))
    outs = [np.asarray(res.results[k]["out"]) for k in range(NCORES)]
    return np.concatenate(outs, axis=-1).astype(np.float32)  # (B, D, N)
